# revision 33
# baseline (speedup 1.0000x reference)
"""Bahdanau additive attention kernel for Trainium2 (8 NeuronCores, SPMD).

Problem: B=32, S=2048, ENC=DEC=ATT=1024 (fp32 inputs)
  u = enc @ U_a                [B,S,A]
  w = dec @ W_a                [B,1,A]
  e = tanh(w + u) @ v_t        [B,S,1]
  align = softmax(e, axis=1)
  context = align^T @ enc      [B,1,E]
  output = tanh([dec, context] @ ffn)   [B,1,D]
  returns (output, context)

Sharding: data-parallel over batch, 4 batches per core, weights replicated.

Per-core pipeline (streaming unit = q: 4 s-tiles = 512 seq positions):
  Pool SWDGE queue : cast loads only (weights fp32->bf16, enc fp32->bf16)
  DVE              : enc bf16 -> fp8 cast; U8 = U*256 fp8 cast; reciprocal
  SP HWDGE         : enc fp8 xbar transposes (fp8 pairs viewed as u16)
  ACT HWDGE        : expe/catT transposes + DRAM stores
  PE               : u-matmuls (fp8 DoubleRow), e-matmuls ([16,128] psum
                     layout: s-tile on partitions), ctx (bf16), ffn
  ACT              : tanh(u/256 + wT) fused, exp with accum, copies

The fp8 transpose packs two consecutive-e fp8 values in one u16 so the
xbar (16-bit only) moves half the bytes; U8 is loaded with the matching
(kb p c) row pairing so DoubleRow contracts the pairs directly.

Software pipelining: ctx(b) + esum matmuls are emitted after u(b+1,q0)
in PE program order so PE never head-of-line blocks on softmax.
"""

import numpy as np
import ml_dtypes

import concourse.bass as bass
import concourse.mybir as mybir
import concourse.tile as tile
from concourse import bacc
from concourse.bass_utils import run_bass_kernel_spmd

F32 = mybir.dt.float32
BF16 = mybir.dt.bfloat16
FP8 = mybir.dt.float8e4
AF = mybir.ActivationFunctionType
DR = mybir.MatmulPerfMode.DoubleRow

U_SCALE = 256.0

B, S, E, A, D = 32, 2048, 1024, 1024, 1024
NCORES = 8
NB = B // NCORES          # 4 batches per core
P = 128
KE = E // P               # 8 e-chunks (128 each)
KB = 4                    # e-pair blocks (256 e-values each) for DoubleRow
MA = A // P               # 8 output chunks over att dim
KD = D // P               # 8 contraction chunks over dec dim
ST = S // P               # 16 s-tiles per batch
NQ = 4                    # streaming units per batch
TQ = ST // NQ             # 4 s-tiles per unit (512 seq)
SQ = TQ * P               # 512 seq per unit
N512 = 512


def _build_kernel_body(tc, repeat=1):
    nc = tc.nc
    enc = nc.dram_tensor("enc", [NB, S, E], F32, kind="ExternalInput")
    dec = nc.dram_tensor("dec", [NB, D], F32, kind="ExternalInput")
    U_a = nc.dram_tensor("U_a", [E, A], F32, kind="ExternalInput")
    W_a = nc.dram_tensor("W_a", [D, A], F32, kind="ExternalInput")
    v_t = nc.dram_tensor("v_t", [A, 1], F32, kind="ExternalInput")
    ffn = nc.dram_tensor("ffn", [D + E, D], F32, kind="ExternalInput")
    out = nc.dram_tensor("out", [NB, D], F32, kind="ExternalOutput")
    ctx_out = nc.dram_tensor("ctx_out", [NB, E], F32, kind="ExternalOutput")
    for _ in range(repeat):
        _build_once(tc, enc, dec, U_a, W_a, v_t, ffn, out, ctx_out)


def _build_once(tc, enc, dec, U_a, W_a, v_t, ffn, out, ctx_out):
    nc = tc.nc
    # s relabeled so each partition reads 4 CONSECUTIVE dram rows (1 big
    # descriptor instead of 4): s = q*512 + p*4 + t. The relabeling flows
    # consistently through u/e/softmax/ctx (softmax is order-invariant and
    # every consumer uses the same tiling), so results are unchanged.
    enc_r = enc.rearrange("b (q p t) e -> b p q t e", q=NQ, p=P, t=TQ)
    # U rows paired (consecutive e) to match the fp8-in-u16 transpose:
    # U_sb[p, (kb c), a] = U[kb*256 + 2p + c, a]
    U_r = U_a.rearrange("(kb p c) a -> p kb c a", kb=KB, p=P, c=2)
    W_r = W_a.rearrange("(k p) a -> p k a", p=P)

    with (
        tc.tile_pool(name="weights", bufs=1) as weights,
        tc.tile_pool(name="big", bufs=1) as big,
        tc.tile_pool(name="enc_nat", bufs=6) as enc_nat_pool,
        tc.tile_pool(name="enc8", bufs=3) as enc8_pool,
        tc.tile_pool(name="encT8", bufs=6) as encT8_pool,
        tc.tile_pool(name="tanhp", bufs=3) as tanh_pool,
        tc.tile_pool(name="rows", bufs=1) as rows,
        tc.tile_pool(name="rows2", bufs=2) as rows2,
        tc.tile_pool(name="psum_u", bufs=2, space="PSUM") as psum_u,
        tc.tile_pool(name="psum_e", bufs=1, space="PSUM") as psum_e,
        tc.tile_pool(name="psum_c", bufs=1, space="PSUM") as psum_c,
        tc.tile_pool(name="psum_s", bufs=1, space="PSUM") as psum_s,
    ):
        # ---------------- Pool-queue cast loads (issue order matters) ----
        dec16 = rows.tile([16, D], BF16, tag="dec16")
        nc.vector.memset(dec16, 0.0)
        nc.gpsimd.dma_start(out=dec16[0:NB, :], in_=dec[:, :])

        def load_enc(b, q):
            nat = enc_nat_pool.tile([P, TQ, E], BF16, name=f"nat_{b}_{q}",
                                    tag="enc_nat")
            nc.gpsimd.dma_start(out=nat, in_=enc_r[b, :, q, :, :])
            return nat

        v_sb = weights.tile([P, MA], BF16)
        nc.gpsimd.dma_start(out=v_sb, in_=v_t.rearrange("(m p) one -> p (m one)", p=P))
        nat00 = load_enc(0, 0)
        # weight loads split into <=512-descriptor DMAs: a bigger one stalls
        # the SWDGE prep ring (1024 descriptors) and blocks the enc stream
        U_sb = weights.tile([P, KB, 2, A], BF16)
        nc.gpsimd.dma_start(out=U_sb[:, 0:2, :, :], in_=U_r[:, 0:2, :, :])
        nc.gpsimd.dma_start(out=U_sb[:, 2:4, :, :], in_=U_r[:, 2:4, :, :])
        nat01 = load_enc(0, 1)
        W_sb = big.tile([P, KD, A], BF16, tag="big")
        nc.gpsimd.dma_start(out=W_sb[:, 0:4, :], in_=W_r[:, 0:4, :])
        nat02 = load_enc(0, 2)
        nc.gpsimd.dma_start(out=W_sb[:, 4:8, :], in_=W_r[:, 4:8, :])
        nat03 = load_enc(0, 3)
        nat_pre = {(0, 0): nat00, (0, 1): nat01, (0, 2): nat02, (0, 3): nat03}

        # ---------------- small shared tiles ----------------
        # catT[p, c, j] = cat[j, c*128+p] ; c 0..7 dec, 8..15 ctx
        catT = weights.tile([P, 2 * KE, 16], BF16)
        nc.scalar.dma_start(out=catT[:, 0:KE, :], in_=dec16, transpose=True)
        ctx16 = rows.tile([16, E], BF16, tag="ctx16")
        nc.vector.memset(ctx16, 0.0)
        id1 = weights.tile([1, 1], F32)
        nc.vector.memset(id1, 1.0)

        # U8[p, kb, c, a] = U_sb[p, (kb c), a] * 256, fp8
        U8 = weights.tile([P, KB, 2, A], FP8)
        for h in range(2):
            nc.vector.tensor_scalar_mul(
                U8[:, 2 * h : 2 * h + 2, :, :].rearrange("p k c a -> p (k c a)"),
                U_sb[:, 2 * h : 2 * h + 2, :, :].rearrange("p k c a -> p (k c a)"),
                U_SCALE,
            )

        # wT[p, m, b] = w[b, m*128+p] = sum_d W[d, m*128+p] dec[b, d]
        wT_ps = psum_c.tile([P, MA, NB], F32, tag="cvec")
        for m in range(MA):
            for k in range(KD):
                nc.tensor.matmul(
                    wT_ps[:, m, :],
                    lhsT=W_sb[:, k, m * P : (m + 1) * P],
                    rhs=catT[:, k, 0:NB],
                    start=(k == 0),
                    stop=(k == KD - 1),
                )
        wT = weights.tile([P, MA, NB], F32)
        nc.scalar.copy(wT, wT_ps)

        # ffn reuses W_sb's slot once W_a is consumed (loaded after batch 1
        # enc loads are queued; only needed at the very end)
        ffn_sb = None

        # ---------------- per-unit build helpers ----------------
        def build_unit_transpose(b, q, nat):
            """bf16 -> fp8 cast then u16-pair xbar transpose."""
            e8 = enc8_pool.tile([P, TQ * E], FP8, name=f"e8_{b}_{q}", tag="e8")
            nc.vector.tensor_copy(e8, nat.rearrange("p t e -> p (t e)"))
            # transpose u16 view: out[p', c, j] = src_u16[j, c*128+p']
            # c = t*4 + kb ; value = enc fp8 pair (e = 2(kb*128+p')+{0,1})
            eT = encT8_pool.tile([P, 4 * TQ, P], mybir.dt.uint16,
                                 name=f"eT_{b}_{q}", tag="encT8")
            nc.sync.dma_start(
                out=eT, in_=e8.bitcast(mybir.dt.uint16), transpose=True
            )
            return eT

        def build_u_block(b, q, eT, e_ps, defer_e=False):
            """u matmuls (fp8 DR) + fused tanh + e-matmuls for unit q.

            With defer_e the e-matmuls are returned as a thunk so the PE
            queue is not blocked on the w/v weight loads at startup."""
            # rhs[kb] = [p, c, t, j] fp8 view of eT
            rhs_all = eT[:, :, :].bitcast(FP8).rearrange(
                "p (t k) (j c) -> p k c t j", k=KB, c=2
            )
            n = q % 2  # 512-block within the half's e_ps row
            ths = []
            for m in range(MA):
                u_ps = psum_u.tile([P, SQ], F32, name="u_ps", tag="u")
                for kb in range(KB):
                    nc.tensor.matmul(
                        u_ps,
                        lhsT=U8[:, kb, :, m * P : (m + 1) * P],
                        rhs=rhs_all[:, kb],
                        start=(kb == 0),
                        stop=(kb == KB - 1),
                        perf_mode=DR,
                    )
                th = tanh_pool.tile([P, SQ], BF16, name="th", tag="th")
                nc.scalar.activation(
                    th, u_ps, AF.Tanh,
                    bias=wT[:, m, b : b + 1],
                    scale=1.0 / U_SCALE,
                )
                ths.append(th)
                if not defer_e:
                    nc.tensor.matmul(
                        e_ps[:, n * N512 : (n + 1) * N512],
                        lhsT=v_sb[:, m : m + 1],
                        rhs=th,
                        start=(m == 0),
                        stop=(m == MA - 1),
                    )
            if not defer_e:
                return None

            def emit_e():
                for m in range(MA):
                    nc.tensor.matmul(
                        e_ps[:, n * N512 : (n + 1) * N512],
                        lhsT=v_sb[:, m : m + 1],
                        rhs=ths[m],
                        start=(m == 0),
                        stop=(m == MA - 1),
                    )

            return emit_e

        def build_exp_q(b, q, e_ps, expe, esum4):
            """exp + accum for one quarter (into the [1,S] expe row)."""
            nc.scalar.activation(
                expe[:, q * SQ : (q + 1) * SQ],
                e_ps[:, (q % 2) * N512 : (q % 2 + 1) * N512],
                AF.Exp,
                accum_out=esum4[:, q : q + 1],
            )

        def build_ctx(b, nats, esum4, expe):
            """expe PE-transposes + esum reduce + ctx matmuls + copy-out."""
            expeT_ps = psum_s.tile([P, ST], F32, name=f"expeT_{b}", tag="eT")
            for tg in range(ST):
                nc.tensor.transpose(
                    expeT_ps[:, tg : tg + 1],
                    expe[:, tg * P : (tg + 1) * P],
                    id1,
                )
            expe_cols = rows2.tile([P, ST], BF16, name=f"expec_{b}",
                                   tag="expe_cols")
            nc.vector.tensor_copy(expe_cols, expeT_ps)
            esum = rows2.tile([1, 1], F32, name=f"esumt_{b}", tag="esumt")
            nc.vector.tensor_reduce(esum, esum4, mybir.AxisListType.X,
                                    mybir.AluOpType.add)
            rsum = rows2.tile([1, 1], F32, name=f"rsum_{b}", tag="rsum")
            nc.vector.reciprocal(rsum, esum)
            ctx_ps = psum_c.tile([1, E], F32, name=f"ctx_ps_{b}", tag="cvec")
            for tg in range(ST):
                for n in range(2):
                    nc.tensor.matmul(
                        ctx_ps[:, n * N512 : (n + 1) * N512],
                        lhsT=expe_cols[:, tg : tg + 1],
                        rhs=nats[tg // TQ][:, tg % TQ, n * N512 : (n + 1) * N512],
                        start=(tg == 0),
                        stop=(tg == ST - 1),
                    )
            ctx_row = rows2.tile([1, E], F32, name=f"ctx_row_{b}", tag="ctx_row")
            nc.scalar.activation(ctx_row, ctx_ps, AF.Copy, scale=rsum)
            nc.scalar.dma_start(out=ctx_out[b : b + 1, :], in_=ctx_row)
            ctx_row16 = rows2.tile([1, E], BF16, name=f"ctx_row16_{b}",
                                   tag="ctx_row16")
            nc.scalar.copy(ctx_row16, ctx_row)
            nc.scalar.dma_start(out=ctx16[b : b + 1, :], in_=ctx_row16)

        # ---------------- main pipeline ----------------
        pending = None  # (b, nats, esum4, expe16) awaiting ctx emission
        for b in range(NB):
            nats = []
            expe = rows2.tile([1, S], F32, name=f"expe_{b}", tag="expe")
            esum4 = rows2.tile([1, NQ], F32, name=f"esum4_{b}", tag="esum4")
            e_ps = None
            deferred = []
            for q in range(NQ):
                if q % 2 == 0:
                    # flush deferred e-matmuls + exps before the psum slot
                    # is recycled for the next half
                    for dq, t, dps in deferred:
                        t()
                        build_exp_q(b, dq, dps, expe, esum4)
                    deferred = []
                    e_ps = psum_e.tile([1, S // 2], F32,
                                       name=f"e_ps_{b}_{q // 2}", tag="e")
                if (b, q) in nat_pre:
                    nat = nat_pre[(b, q)]
                else:
                    nat = load_enc(b, q)
                nats.append(nat)
                eT = build_unit_transpose(b, q, nat)
                # batch 0 q0/q1: keep the PE queue free of e-matmuls until
                # the W/v loads have certainly landed
                defer = b == 0 and q < 2
                thunk = build_u_block(b, q, eT, e_ps, defer_e=defer)
                if thunk is not None:
                    deferred.append((q, thunk, e_ps))
                else:
                    build_exp_q(b, q, e_ps, expe, esum4)
                if pending is not None and q == 0:
                    # ctx of the previous batch lands behind u(b, q0) on PE
                    build_ctx(*pending)
                    pending = None
            pending = (b, nats, esum4, expe)
        ffn_sb = big.tile([P, 2 * KE, D], BF16, tag="big")
        ffn_r = ffn.rearrange("(k p) d -> p k d", p=P)
        for c in range(4):
            nc.gpsimd.dma_start(
                out=ffn_sb[:, c * 4 : (c + 1) * 4, :],
                in_=ffn_r[:, c * 4 : (c + 1) * 4, :],
            )
        build_ctx(*pending)

        # ---------------- final ffn (all batches at once) ----------------
        nc.scalar.dma_start(out=catT[:, KE : 2 * KE, :], in_=ctx16,
                            transpose=True)
        out_ps = psum_c.tile([NB, D], F32, tag="cvec")
        for c in range(2 * KE):
            for n in range(2):
                nc.tensor.matmul(
                    out_ps[:, n * N512 : (n + 1) * N512],
                    lhsT=catT[:, c, 0:NB],
                    rhs=ffn_sb[:, c, n * N512 : (n + 1) * N512],
                    start=(c == 0),
                    stop=(c == 2 * KE - 1),
                )
        out_sb = weights.tile([NB, D], F32)
        nc.scalar.activation(out_sb, out_ps, AF.Tanh)
        nc.scalar.dma_start(out=out[:, :], in_=out_sb)


_NC_CACHE = None


def _get_nc(repeat=1):
    global _NC_CACHE
    if repeat != 1:
        nc = bacc.Bacc(None, target_bir_lowering=False)
        with tile.TileContext(nc) as tc:
            _build_kernel_body(tc, repeat=repeat)
        nc.compile()
        return nc
    if _NC_CACHE is None:
        nc = bacc.Bacc(None, target_bir_lowering=False)
        with tile.TileContext(nc) as tc:
            _build_kernel_body(tc)
        nc.compile()
        _NC_CACHE = nc
    return _NC_CACHE


def kernel(encoder_hidden_states, decoder_hidden_state, U_a, W_a, v_t, ffn,
           _trace=False):
    enc = np.ascontiguousarray(np.asarray(encoder_hidden_states, dtype=np.float32))
    dec = np.ascontiguousarray(
        np.asarray(decoder_hidden_state, dtype=np.float32).reshape(B, D)
    )
    U = np.ascontiguousarray(np.asarray(U_a, dtype=np.float32))
    W = np.ascontiguousarray(np.asarray(W_a, dtype=np.float32))
    v = np.ascontiguousarray(np.asarray(v_t, dtype=np.float32))
    F = np.ascontiguousarray(np.asarray(ffn, dtype=np.float32))

    nc = _get_nc()
    in_maps = []
    for c in range(NCORES):
        sl = slice(c * NB, (c + 1) * NB)
        in_maps.append(
            {
                "enc": enc[sl],
                "dec": dec[sl],
                "U_a": U,
                "W_a": W,
                "v_t": v,
                "ffn": F,
            }
        )
    res = run_bass_kernel_spmd(nc, in_maps, core_ids=list(range(NCORES)),
                               trace=_trace)

    output = np.empty((B, 1, D), dtype=np.float32)
    context = np.empty((B, 1, E), dtype=np.float32)
    for c in range(NCORES):
        sl = slice(c * NB, (c + 1) * NB)
        output[sl, 0, :] = res.results[c]["out"]
        context[sl, 0, :] = res.results[c]["ctx_out"]
    if _trace:
        return (output, context), res
    return (output, context)


if __name__ == "__main__":
    import reference

    inputs = {k: np.asarray(v) for k, v in reference.setup_inputs().items()}
    (o, c) = kernel(**inputs)
    print("output", o.shape, o.dtype, "context", c.shape, c.dtype)
